# revision 18
# baseline (speedup 1.0000x reference)
"""Trainium2 Bass kernel for nn_BehaviorSnake: one CA step on a [B,C,H,W] world.

Sharding: batch-parallel, world[b] -> core b (B == n_cores == 8).

Host staging: the one-hot/dir channels are exact in bf16, so the host packs
snake/wall/empty/dir as bf16 planes (half the HBM traffic of f32) and upcasts
the bf16 outputs back to f32.  rm/re/energy stay f32 on device: the rand
thresholds (0.1/0.05/0.5) and energy-0.1 must be computed in f32 to match the
reference bit-exactly where it matters; rounding the exact f32 *result* to
bf16 only adds <=2^-9 relative error, inside the 2e-2 gate.  The always-zero
channels 4,5,8,9 and the wall passthrough are assembled host-side.

Device algorithm (per core, planes are [128 part, 4, 512] = 512x512):
 - DS = (dir+1)*snake in {0..4} encodes "snake cell with angle a" as a single
   plane; per-angle masks are single compares (DS==a+1), reset handling
   multiplies DS by (prev-targets==0) gates.
 - Move targets are VALUED shifts V_a=(a+1)*mask so dbs accumulates by max and
   dir_came = max(shifts)-1 (later angle wins = larger value, matching the
   reference's last-write-wins).
 - W-direction torus shifts fold into consumer read APs (main + 1-col wrap
   ops); H-direction shifts are SBUF->SBUF partition-offset DMAs (engine-free
   on the HWDGE queue) + 1-row wrap DMAs.
 - Engine balance: DVE does the cheap bf16 TT/TSP work, GPSIMD (Pool) takes
   fused scalar_tensor_tensor ops ((x op s) op y in one pass), ScalarE (Act)
   takes single-src affine/sign/cast ops, DMA queues take the shifts.
 - Every plane op is emitted as two half-plane ops (Split proxies) so
   dependent chains pipeline at half-plane granularity.
 - Short-lived planes share one 16-buffer ring (tag "r"); emission order keeps
   each tile's readers within 16 subsequent ring allocations.

Channels: 0=EMPTY 1=WALL 2=ACID 3=SNAKE 6=DIR 7=ENERGY; 4,5,8,9 always zero.
Device output planes: 0=EMPTY 1=ACID 2=SNAKE 3=DIR 4=ENERGY (bf16).
"""

import numpy as np
import ml_dtypes

import concourse.bacc as bacc
import concourse.mybir as mybir
import concourse.tile as tile
from concourse import bass_utils

OP = mybir.AluOpType
AF = mybir.ActivationFunctionType
DTB = mybir.dt.bfloat16
DTF = mybir.dt.float32
DTU8 = mybir.dt.uint8

B, C, H, W = 8, 10, 512, 512
NCORES = 8


def snake_body(tc, outs, ins):
    nc = tc.nc
    wb = ins["wb"]      # [4,H,W] bf16: 0=snake 1=wall 2=empty 3=dir
    en = ins["en"]      # [H,W] f32
    rm = ins["rm"]      # [H,W] f32
    re = ins["re"]      # [H,W] f32
    out = outs["out"]   # [4,H,W] bf16: empty, snake, dir, energy

    def rp(x):
        return x.rearrange("(t p) w -> p t w", p=128)

    from concourse.bass import AP as _AP

    _SPLIT = {
        "tensor_mul",
        "tensor_add",
        "tensor_sub",
        "tensor_max",
        "tensor_copy",
        "tensor_scalar",
        "tensor_single_scalar",
        "tensor_scalar_mul",
        "tensor_scalar_add",
        "scalar_tensor_tensor",
        "copy_predicated",
        "copy",
        "add",
        "activation",
        "tensor_tensor",
        "dma_start",
    }

    class Split:
        def __init__(self, eng):
            self._e = eng

        def __getattr__(self, name):
            f = getattr(self._e, name)
            if name not in _SPLIT:
                return f

            def g(*args, **kw):
                did = False

                def cut(x, sl):
                    nonlocal did
                    if (
                        isinstance(x, _AP)
                        and len(x.shape) == 3
                        and x.shape[1] == 4
                    ):
                        did = True
                        return x[:, sl]
                    return x

                for sl in (slice(0, 2), slice(2, 4)):
                    a2 = [cut(x, sl) for x in args]
                    k2 = {k: cut(v, sl) for k, v in kw.items()}
                    f(*a2, **k2)
                    if not did:
                        return

            return g

    V = Split(nc.vector)
    P = Split(nc.gpsimd)
    A = Split(nc.scalar)
    SY = Split(nc.sync)
    AS = A  # scalar.dma_start = second HWDGE ring (qActDynamicHW)

    SHP = [128, 4, 512]

    with (
        tc.tile_pool(name="mp", bufs=1) as mp,
        tc.tile_pool(name="shp", bufs=2) as shp,
    ):
        def rtile(nm):
            return shp.tile(SHP, DTB, tag="r", name=nm, bufs=16)

        def hshift(nm, src, up, q=None):
            # Torus roll along H via SBUF->SBUF DMA: bulk partition-offset copy
            # + wrap row from the neighboring block (engine-free on HWDGE).
            q = q or SY
            d = shp.tile(SHP, DTB, tag="hscopy", name=nm, bufs=6)
            if up:  # out[h] = in[h-1]
                q.dma_start(out=d[1:128, :, :], in_=src[0:127, :, :])
                q.dma_start(out=d[0:1, 1:4, :], in_=src[127:128, 0:3, :])
                q.dma_start(out=d[0:1, 0:1, :], in_=src[127:128, 3:4, :])
            else:  # out[h] = in[h+1]
                q.dma_start(out=d[0:127, :, :], in_=src[1:128, :, :])
                q.dma_start(out=d[127:128, 0:3, :], in_=src[0:1, 1:4, :])
                q.dma_start(out=d[127:128, 3:4, :], in_=src[0:1, 0:1, :])
            return d

        def wshift_dma(nm, src, plus, q=None):
            # Torus roll along W via SBUF->SBUF DMA (free-dim offset).
            q = q or SY
            d = shp.tile(SHP, DTB, tag="wscopy", name=nm, bufs=2)
            if plus:  # out[w] = in[w+1]
                q.dma_start(out=d[:, :, 0:511], in_=src[:, :, 1:512])
                q.dma_start(out=d[:, :, 511:512], in_=src[:, :, 0:1])
            else:  # out[w] = in[w-1]
                q.dma_start(out=d[:, :, 1:512], in_=src[:, :, 0:511])
                q.dma_start(out=d[:, :, 0:1], in_=src[:, :, 511:512])
            return d

        # ---- loads (HWDGE, engine-free), critical-path first.
        Sb = mp.tile(SHP, DTB, name="Sb")
        Db = mp.tile(SHP, DTB, name="Db")
        for q in range(4):
            for src_c, t in ((0, Sb), (3, Db)):
                SY.dma_start(
                    out=t[:, q : q + 1, :], in_=rp(wb[src_c])[:, q : q + 1, :]
                )
        Wlb = mp.tile(SHP, DTB, name="Wlb")
        SY.dma_start(out=Wlb[:, :, :], in_=rp(wb[1]))
        RmF = mp.tile(SHP, DTF, name="RmF")
        SY.dma_start(out=RmF[:, :, :], in_=rp(rm))
        ReF = mp.tile(SHP, DTF, name="ReF")
        AS.dma_start(out=ReF[:, :, :], in_=rp(re))
        EnF = mp.tile(SHP, DTF, name="EnF")
        SY.dma_start(out=EnF[:, :, :], in_=rp(en))
        E0b = mp.tile(SHP, DTB, name="E0b")
        AS.dma_start(out=E0b[:, :, :], in_=rp(wb[2]))

        # ---- angle-0/1 critical chain ----
        dp1 = rtile("dp1")
        A.activation(dp1[:, :, :], Db[:, :, :], AF.Copy, bias=1.0, scale=1.0)
        DS = rtile("DS")
        V.tensor_mul(DS[:, :, :], dp1[:, :, :], Sb[:, :, :])
        V0 = rtile("V0")
        V.tensor_single_scalar(V0[:, :, :], DS[:, :, :], 1.0, OP.is_equal)
        V1 = rtile("V1")
        V.tensor_scalar(V1[:, :, :], DS[:, :, :], 2.0, 2.0, OP.is_equal, OP.mult)
        R1 = hshift("R1", V1, up=True)
        ws0 = mp.tile(SHP, DTB, name="ws0")
        V.tensor_add(ws0[:, :, :], Wlb[:, :, :], Sb[:, :, :])
        shw1 = hshift("shw1", ws0, up=False)

        # preamble compares (off critical path, fill engine gaps early)
        t_acc = mp.tile(SHP, DTB, name="t_acc")
        V.tensor_single_scalar(t_acc[:, :, :], RmF[:, :, :], 0.1, OP.is_lt)
        q2 = mp.tile(SHP, DTB, name="q2")
        V.tensor_scalar(q2[:, :, :], ReF[:, :, :], 0.5, -2.0, OP.is_lt, OP.mult)
        l05 = mp.tile(SHP, DTB, name="l05")
        V.tensor_single_scalar(l05[:, :, :], ReF[:, :, :], 0.05, OP.is_lt)
        notW = mp.tile(SHP, DTB, name="notW")
        A.activation(notW[:, :, :], Wlb[:, :, :], AF.Copy, bias=1.0, scale=-1.0)
        epos = mp.tile(SHP, DTB, name="epos")
        A.activation(epos[:, :, :], EnF[:, :, :], AF.Sign)
        em = mp.tile(SHP, DTB, name="em")  # exact f32 compute, bf16 rounding
        A.activation(em[:, :, :], EnF[:, :, :], AF.Copy, bias=-0.1, scale=1.0)
        Enb = mp.tile(SHP, DTB, name="Enb")
        A.activation(Enb[:, :, :], EnF[:, :, :], AF.Copy)

        # blocked&target products; W-shifts fold into read APs (R0 = V0
        # shifted w-1, so bd0(c) = ws0(c+1)*V0(c-1): 3 pieces).
        bd0 = rtile("bd0")
        V.tensor_mul(bd0[:, :, 1:511], ws0[:, :, 2:512], V0[:, :, 0:510])
        V.tensor_mul(bd0[:, :, 0:1], ws0[:, :, 1:2], V0[:, :, 511:512])
        V.tensor_mul(bd0[:, :, 511:512], ws0[:, :, 0:1], V0[:, :, 510:511])
        V.tensor_max(t_acc[:, :, :], t_acc[:, :, :], bd0[:, :, :])
        rst1 = rtile("rst1")
        V.tensor_mul(rst1[:, :, 1:512], V0[:, :, 0:511], R1[:, :, 1:512])
        V.tensor_mul(rst1[:, :, 0:1], V0[:, :, 511:512], R1[:, :, 0:1])
        nR1 = rtile("nR1")
        V.tensor_single_scalar(nR1[:, :, :], rst1[:, :, :], 0.0, OP.is_equal)
        bd1 = rtile("bd1")
        P.tensor_mul(bd1[:, :, :], shw1[:, :, :], R1[:, :, :])
        V.tensor_max(t_acc[:, :, :], t_acc[:, :, :], bd1[:, :, :])

        # ---- angle 2 (reset gate 1) ----
        DS2 = rtile("DS2")
        V.tensor_mul(DS2[:, :, :], DS[:, :, :], nR1[:, :, :])
        V2 = rtile("V2")
        V.tensor_scalar(V2[:, :, :], DS2[:, :, :], 3.0, 3.0, OP.is_equal, OP.mult)
        ws2p = rtile("ws2p")  # wall|snake2 in nonzero-sense: Wlb + DS2
        V.tensor_add(ws2p[:, :, :], Wlb[:, :, :], DS2[:, :, :])
        bd2 = rtile("bd2")  # bd2(c) = ws2p(c-1)*V2(c+1)
        V.tensor_mul(bd2[:, :, 1:511], ws2p[:, :, 0:510], V2[:, :, 2:512])
        V.tensor_mul(bd2[:, :, 0:1], ws2p[:, :, 511:512], V2[:, :, 1:2])
        V.tensor_mul(bd2[:, :, 511:512], ws2p[:, :, 510:511], V2[:, :, 0:1])
        V.tensor_max(t_acc[:, :, :], t_acc[:, :, :], bd2[:, :, :])

        # trail union via maxes (the V_a are disjoint)
        trailT = rtile("trailT")
        V.tensor_max(trailT[:, :, :], V0[:, :, :], V1[:, :, :])
        V.tensor_max(trailT[:, :, :], trailT[:, :, :], V2[:, :, :])

        dbs01 = rtile("dbs01")
        V.tensor_max(dbs01[:, :, 1:512], V0[:, :, 0:511], R1[:, :, 1:512])
        V.tensor_max(dbs01[:, :, 0:1], V0[:, :, 511:512], R1[:, :, 0:1])

        # ---- angle 3 (reset gate 2) ----
        rst2 = rtile("rst2")  # R2(c) = V2(c+1)
        V.tensor_mul(rst2[:, :, 0:511], dbs01[:, :, 0:511], V2[:, :, 1:512])
        V.tensor_mul(rst2[:, :, 511:512], dbs01[:, :, 511:512], V2[:, :, 0:1])
        nR2 = rtile("nR2")
        V.tensor_single_scalar(nR2[:, :, :], rst2[:, :, :], 0.0, OP.is_equal)
        DS3 = rtile("DS3")
        V.tensor_mul(DS3[:, :, :], DS2[:, :, :], nR2[:, :, :])
        V3 = rtile("V3")
        V.tensor_scalar(V3[:, :, :], DS3[:, :, :], 4.0, 4.0, OP.is_equal, OP.mult)
        R3 = hshift("R3", V3, up=False)
        ws3p = rtile("ws3p")
        V.tensor_add(ws3p[:, :, :], Wlb[:, :, :], DS3[:, :, :])
        shw3 = hshift("shw3", ws3p, up=True)
        bd3 = rtile("bd3")
        P.tensor_mul(bd3[:, :, :], shw3[:, :, :], R3[:, :, :])
        V.tensor_max(t_acc[:, :, :], t_acc[:, :, :], bd3[:, :, :])

        # ---- dbs / trail / dir_came ----
        V.tensor_max(trailT[:, :, :], trailT[:, :, :], V3[:, :, :])
        dbs012 = rtile("dbs012")
        V.tensor_max(dbs012[:, :, 0:511], dbs01[:, :, 0:511], V2[:, :, 1:512])
        V.tensor_max(dbs012[:, :, 511:512], dbs01[:, :, 511:512], V2[:, :, 0:1])
        dbsM = mp.tile(SHP, DTB, name="dbsM")
        V.tensor_max(dbsM[:, :, :], dbs012[:, :, :], R3[:, :, :])
        tr = rtile("tr")
        V.tensor_single_scalar(tr[:, :, :], trailT[:, :, :], 0.0, OP.not_equal)
        nottrail = mp.tile(SHP, DTB, name="nottrail")
        A.activation(nottrail[:, :, :], tr[:, :, :], AF.Copy, bias=1.0, scale=-1.0)

        # turned = mod4(dirc + 1 - 2*(re<0.5)), branch-free; x5a = dirc+1
        x5a = rtile("x5a")
        V.tensor_single_scalar(x5a[:, :, :], dbsM[:, :, :], 1.0, OP.max)
        dirc = mp.tile(SHP, DTB, name="dirc")
        V.tensor_single_scalar(dirc[:, :, :], x5a[:, :, :], -1.0, OP.add)
        x5 = rtile("x5")
        V.tensor_add(x5[:, :, :], x5a[:, :, :], q2[:, :, :])
        c1x = rtile("c1x")
        V.tensor_scalar(c1x[:, :, :], x5[:, :, :], 0.0, 4.0, OP.is_lt, OP.mult)
        y4 = rtile("y4")
        V.tensor_add(y4[:, :, :], x5[:, :, :], c1x[:, :, :])
        c2x = rtile("c2x")
        V.tensor_scalar(c2x[:, :, :], y4[:, :, :], 4.0, -4.0, OP.is_ge, OP.mult)
        turned = mp.tile(SHP, DTB, name="turned")
        V.tensor_add(turned[:, :, :], y4[:, :, :], c2x[:, :, :])

        # ---- element outputs ----
        TE = rtile("TE")
        V.tensor_mul(TE[:, :, :], tr[:, :, :], epos[:, :, :])
        tnE = rtile("tnE")
        V.tensor_sub(tnE[:, :, :], tr[:, :, :], TE[:, :, :])
        dbb = rtile("dbb")
        A.activation(dbb[:, :, :], dbsM[:, :, :], AF.Sign)
        dbW = rtile("dbW")
        V.tensor_mul(dbW[:, :, :], dbb[:, :, :], notW[:, :, :])
        out_S = mp.tile(SHP, DTB, name="out_S")
        V.tensor_max(out_S[:, :, :], dbW[:, :, :], tnE[:, :, :])
        SY.dma_start(out=rp(out[1]), in_=out_S[:, :, :])
        SW = mp.tile(SHP, DTB, name="SW")
        V.tensor_add(SW[:, :, :], out_S[:, :, :], Wlb[:, :, :])
        Su8 = mp.tile(SHP, DTU8, name="Su8")
        A.activation(Su8[:, :, :], out_S[:, :, :], AF.Copy)

        u = rtile("u")
        P.tensor_mul(u[:, :, :], l05[:, :, :], TE[:, :, :])
        wE = rtile("wE")
        P.tensor_add(wE[:, :, :], u[:, :, :], E0b[:, :, :])
        ndbs = rtile("ndbs")
        A.activation(ndbs[:, :, :], dbb[:, :, :], AF.Copy, bias=1.0, scale=-1.0)
        out_E = rtile("out_E")
        P.tensor_mul(out_E[:, :, :], ndbs[:, :, :], wE[:, :, :])
        AS.dma_start(out=rp(out[0]), in_=out_E[:, :, :])

        # ---- in-dir check: acc = SW at turned-direction neighbor ----
        SW2 = hshift("SW2", SW, up=False)
        SW6 = hshift("SW6", SW, up=True, q=AS)
        m0 = rtile("m0")
        V.tensor_single_scalar(m0[:, :, :], turned[:, :, :], 0.0, OP.is_equal)
        tk0 = rtile("tk0")
        V.tensor_mul(tk0[:, :, 0:511], m0[:, :, 0:511], SW[:, :, 1:512])
        V.tensor_mul(tk0[:, :, 511:512], m0[:, :, 511:512], SW[:, :, 0:1])
        m1 = rtile("m1")
        V.tensor_single_scalar(m1[:, :, :], y4[:, :, :], 1.0, OP.is_equal)
        tk1 = rtile("tk1")
        V.tensor_mul(tk1[:, :, :], m1[:, :, :], SW2[:, :, :])
        m2 = rtile("m2")
        V.tensor_single_scalar(m2[:, :, :], y4[:, :, :], 2.0, OP.is_equal)
        tk2 = rtile("tk2")
        V.tensor_mul(tk2[:, :, 1:512], m2[:, :, 1:512], SW[:, :, 0:511])
        V.tensor_mul(tk2[:, :, 0:1], m2[:, :, 0:1], SW[:, :, 511:512])
        m3 = rtile("m3")
        V.tensor_single_scalar(m3[:, :, :], y4[:, :, :], 3.0, OP.is_equal)
        tk3 = rtile("tk3")
        V.tensor_mul(tk3[:, :, :], m3[:, :, :], SW6[:, :, :])
        a01 = rtile("a01")
        V.tensor_add(a01[:, :, :], tk0[:, :, :], tk1[:, :, :])
        a23 = rtile("a23")
        V.tensor_add(a23[:, :, :], tk2[:, :, :], tk3[:, :, :])
        acc = rtile("acc")
        V.tensor_add(acc[:, :, :], a01[:, :, :], a23[:, :, :])
        nacc = rtile("nacc")
        V.tensor_single_scalar(nacc[:, :, :], acc[:, :, :], 0.0, OP.is_equal)
        tUb = rtile("tUb")
        V.tensor_mul(tUb[:, :, :], nacc[:, :, :], t_acc[:, :, :])

        # ---- dir / energy outputs ----
        # out_D = where(nb & tU, turned, where(nb, dirc, Db*keep)); predicating
        # with nb early and nb&tU late removes the serial dirc-CP.
        keepm = rtile("keepm")
        V.tensor_max(keepm[:, :, :], nottrail[:, :, :], out_S[:, :, :])
        out_D = rtile("out_D")
        V.tensor_mul(out_D[:, :, :], Db[:, :, :], keepm[:, :, :])
        nS0 = rtile("nS0")
        A.activation(nS0[:, :, :], Sb[:, :, :], AF.Copy, bias=1.0, scale=-1.0)
        nbb = rtile("nbb")
        P.tensor_mul(nbb[:, :, :], nS0[:, :, :], out_S[:, :, :])
        nb = shp.tile(SHP, DTU8, tag="u8", name="nb", bufs=2)
        A.activation(nb[:, :, :], nbb[:, :, :], AF.Copy)
        V.copy_predicated(out_D[:, :, :], nb[:, :, :], dirc[:, :, :])
        nbtU = rtile("nbtU")
        V.tensor_mul(nbtU[:, :, :], nbb[:, :, :], tUb[:, :, :])
        nbtU8 = shp.tile(SHP, DTU8, tag="u8", name="nbtU8", bufs=2)
        A.activation(nbtU8[:, :, :], nbtU[:, :, :], AF.Copy)
        V.copy_predicated(out_D[:, :, :], nbtU8[:, :, :], turned[:, :, :])
        SY.dma_start(out=rp(out[2]), in_=out_D[:, :, :])

        out_En = mp.tile(SHP, DTB, name="out_En")
        P.tensor_mul(out_En[:, :, :], nottrail[:, :, :], Enb[:, :, :])
        V.copy_predicated(out_En[:, :, :], Su8[:, :, :], em[:, :, :])
        AS.dma_start(out=rp(out[3]), in_=out_En[:, :, :])


_CACHED = None


def build_program():
    global _CACHED
    if _CACHED is not None:
        return _CACHED
    nc = bacc.Bacc("TRN2", target_bir_lowering=False, debug=False, num_devices=NCORES)
    wb_t = nc.dram_tensor("wb", [4, H, W], DTB, kind="ExternalInput").ap()
    en_t = nc.dram_tensor("en", [H, W], DTF, kind="ExternalInput").ap()
    rm_t = nc.dram_tensor("rm", [H, W], DTF, kind="ExternalInput").ap()
    re_t = nc.dram_tensor("re", [H, W], DTF, kind="ExternalInput").ap()
    out_t = nc.dram_tensor("out", [4, H, W], DTB, kind="ExternalOutput").ap()
    with tile.TileContext(nc) as tc:
        snake_body(
            tc,
            {"out": out_t},
            {"wb": wb_t, "en": en_t, "rm": rm_t, "re": re_t},
        )
    nc.compile()
    _CACHED = nc
    return nc


def kernel(**inputs) -> np.ndarray:
    world = np.asarray(inputs["world"], dtype=np.float32)
    rmov = np.ascontiguousarray(
        np.asarray(inputs["rand_movement"], dtype=np.float32)[:, 0]
    )
    rele = np.ascontiguousarray(
        np.asarray(inputs["rand_element"], dtype=np.float32)[:, 0]
    )
    en = np.ascontiguousarray(world[:, 7])
    # bf16 staging of the exactly-representable planes: snake, wall, empty, dir
    wb = np.ascontiguousarray(
        np.stack([world[:, 3], world[:, 1], world[:, 0], world[:, 6]], axis=1).astype(
            ml_dtypes.bfloat16
        )
    )

    nc = build_program()
    in_maps = [
        {"wb": wb[b], "en": en[b], "rm": rmov[b], "re": rele[b]} for b in range(B)
    ]
    res = bass_utils.run_bass_kernel_spmd(nc, in_maps, core_ids=list(range(NCORES)))

    full = np.zeros((B, C, H, W), np.float32)
    full[:, 1] = world[:, 1]  # wall passthrough
    for b in range(B):
        ob = np.asarray(res.results[b]["out"]).astype(np.float32)
        full[b, 0] = ob[0]
        full[b, 3] = ob[1]
        full[b, 6] = ob[2]
        full[b, 7] = ob[3]
    # acid is the remaining one-hot category: 1 - wall - empty - snake
    full[:, 2] = 1.0 - world[:, 1] - full[:, 0] - full[:, 3]
    return full
